# revision 15
# baseline (speedup 1.0000x reference)
"""Single-head causal attention on 8 NeuronCores (batch-parallel), bf16.

x [8, 2048, 1024], Wq/Wk/Wv [1024, 64] -> out [8, 2048, 64].
One batch element per core. The host pre-transposes x to x.T (chunk-major
layout) and casts everything to bf16 (zero-flop marshalling), so the
device does no transposes at all:

  qkT[:,t]   = [Wq|Wk].T @ xT[:,t]      (qT rows 0:64, kT rows 64:128)
  klo[:,s]   = id64.T @ qkT[64:128,s]   (PE identity matmul re-bases kT to
                                         partition 0 — both operands at
                                         base partition 64, out at 0 — so
                                         no SBUF->SBUF rebase DMA exists)
  v[t,:]     = xT[:,t-tile].T @ Wv      (natural [t,h] layout, PE direct)
  weiT[s,t]  = k[s]. q[t]              (lhsT = klo tile, rhs = qT cols)
  pT         = exp(weiT / sqrt(H))      (ACT, f32 psum -> bf16 sbuf,
                                         two s-tiles per instruction)
  out[t,h]   = sum_s pT[s,t] vaug[s,h]  (natural PV; ones column gives
                                         softmax denominators)
  out[t,h]  /= out[t,64]               (DVE reciprocal + scalar mul)

Causality via tile skipping, column-restricted diagonal score matmuls,
and one [128,128] triangular bf16 mask (host-uploaded) on diagonal
blocks.

DMA plan: each of the three queues (scalar/sync/gpsimd) carries a
balanced slice of every x chunk, chained in chunk order; a queue's
chunk-n+1 slice is gated on its own chunk-n slice so all queues stream
continuously at full aggregate bandwidth while chunks complete in
order.  Each slice is split into two sub-DMAs and the projection
accumulates c-tiles in semaphore-arrival order (late tiles last), which
hides most of the ~2us DMA completion-semaphore lag.  Gates that would
otherwise be hoisted by the tile scheduler ahead of the exp stream are
keyed on pT tiles (compute artifacts), with a second gate carrying the
real DMA dependency.  Output DMAs ride the sync queue pinned behind its
x chain; the last chunk's epilogue is split and its output DMA spread
over three queues to shorten the tail.
"""

from contextlib import ExitStack

import ml_dtypes
import numpy as np

import concourse.bass as bass
import concourse.mybir as mybir
import concourse.tile as tile
from concourse import bacc
from concourse.bass_utils import run_bass_kernel_spmd

B, T, C, H = 8, 2048, 1024, 64
P = 128                      # partition tile
NT = T // P                  # 16 row tiles
NC = C // P                  # 8 contraction tiles
CH = 512                     # t-chunk width (psum bank)
NCH = T // CH                # 4 chunks
TPC = CH // P                # 4 t-tiles per chunk
VW = 66                      # vaug row stride: [v(64) | 1 | pad]

BF = mybir.dt.bfloat16
F32 = mybir.dt.float32
BF_NP = ml_dtypes.bfloat16

Exp = mybir.ActivationFunctionType.Exp
Copy = mybir.ActivationFunctionType.Copy

# per-queue sub-dma c-tile splits of each x chunk: {engine: [(lo,hi),...]}
XSPLIT = {
    0: {"scalar": [(0, 1), (1, 2)], "sync": [(2, 4), (4, 5)], "gpsimd": [(5, 7), (7, 8)]},
    1: {"scalar": [(0, 2), (2, 3)], "sync": [(3, 5), (5, 6)], "gpsimd": [(6, 7), (7, 8)]},
    2: {"scalar": [(0, 2), (2, 3)], "sync": [(3, 4), (4, 5)], "gpsimd": [(5, 7), (7, 8)]},
    3: {"scalar": [(0, 1), (1, 2)], "sync": [(2, 4), (4, 5)], "gpsimd": [(5, 7), (7, 8)]},
}


def _c_order(ch):
    """projection accumulation order: first sub-dmas' c-tiles, then seconds'."""
    firsts, seconds = [], []
    for subs in XSPLIT[ch].values():
        firsts.extend(range(subs[0][0], subs[0][1]))
        seconds.extend(range(subs[1][0], subs[1][1]))
    return firsts + seconds


def build_kernel():
    nc = bacc.Bacc(
        "TRN2",
        target_bir_lowering=False,
        debug=False,
        enable_asserts=False,
        num_devices=B,
    )
    xTd = nc.dram_tensor("xT", [NCH, P, NC, CH], BF, kind="ExternalInput").ap()
    wqkd = nc.dram_tensor("Wqk", [P, NC, P], BF, kind="ExternalInput").ap()
    wvd = nc.dram_tensor("Wv", [P, NC, H], BF, kind="ExternalInput").ap()
    # tri[p,j]=1 iff j>=p (cols 0:128); id64 eye in rows 64:128, cols 128:192
    maskd = nc.dram_tensor("mask", [P, P + H], BF, kind="ExternalInput").ap()
    outd = nc.dram_tensor("out", [T, H], F32, kind="ExternalOutput").ap()

    with tile.TileContext(nc) as tc, ExitStack() as ctx:
        const = ctx.enter_context(tc.tile_pool(name="const", bufs=1))
        persist = ctx.enter_context(tc.tile_pool(name="persist", bufs=1))
        pt_p = ctx.enter_context(tc.tile_pool(name="pt", bufs=20))
        ost_p = ctx.enter_context(tc.tile_pool(name="ost", bufs=4))
        rc_p = ctx.enter_context(tc.tile_pool(name="rc", bufs=4))
        proj_ps = ctx.enter_context(tc.tile_pool(name="projps", bufs=2, space="PSUM"))
        wei_ps = ctx.enter_context(tc.tile_pool(name="weips", bufs=2, space="PSUM"))
        o_ps_p = ctx.enter_context(tc.tile_pool(name="ops", bufs=2, space="PSUM"))

        # PE p-state warmup: matmuls on a zeroed scratch tile (no DMA
        # dependency) keep the tensor engine running during the x DMA wait so
        # the clock has ramped to 2.4 GHz (and stays there) until the first
        # real projection is ready.
        garbage = const.tile([P, CH], BF, tag="garbage")
        nc.vector.memset(garbage, 0.0)
        warm_ps = proj_ps.tile([P, CH], F32, tag="ps")
        for _ in range(17):
            nc.tensor.matmul(warm_ps, garbage[:, 0:P], garbage, start=True, stop=True)

        xT = persist.tile([P, NC, T], BF, tag="xT")  # x.T: [c, t]
        wqk = const.tile([P, NC, P], BF, tag="wqk")
        wv = const.tile([P, NC, H], BF, tag="wv")
        mask = const.tile([P, P + H], BF, tag="mask")  # [tri | id64(rows 64:)]
        tri = mask[:, 0:P]
        id64 = mask[64:P, P : P + H]
        gate_scr = const.tile([1, 8], BF, tag="gate_scr")  # gate dsts

        def x_subs(eng_name, ch):
            c0 = ch * CH
            for lo, hi in XSPLIT[ch][eng_name]:
                yield xT[:, lo:hi, c0 : c0 + CH], xTd[ch][:, lo:hi, :]

        def x_part_tail(eng_name, ch):
            """last element of a queue's chunk-ch slice (sem-carrying read)"""
            hi = XSPLIT[ch][eng_name][-1][1]
            return xT[0:1, hi - 1, ch * CH + CH - 1 : ch * CH + CH]

        def _gate_elems(eng_name, ch):
            # one element inside EACH sub-dma's dst region: the gate copy
            # writes these, giving every sub-dma a real WAW dependency on the
            # gate — the scheduler cannot hoist the dma ahead of it, and the
            # dma's descriptor enqueue waits for the gate's semaphore.
            c0 = ch * CH
            return [xT[0:1, lo, c0 : c0 + 1] for lo, _ in XSPLIT[ch][eng_name]]

        def scalar_x(ch, key=None):
            # gate on scalar's own previous slice; if a pT element is given,
            # a first copy keyed on it pins the gate chain's position in the
            # scheduled exp stream (the scheduler's DMA model is optimistic
            # and would otherwise schedule the gate ahead of earlier exps,
            # where its real wait would stall them).
            g = _gate_elems("scalar", ch)
            if key is not None:
                nc.scalar.activation(g[0], key, Copy)
            nc.scalar.activation(g[0], x_part_tail("scalar", ch - 1), Copy)
            for el in g[1:]:
                nc.scalar.activation(el, g[0], Copy)
            for dst, src in x_subs("scalar", ch):
                nc.scalar.dma_start(dst, src)

        def sync_x(ch):
            # sync cannot run compute ops; 1-element sbuf->sbuf dmas carry
            # the read dependency on the previous slice and write into each
            # sub-dma's dst region (WAW gates the real dmas at issue).
            tail = x_part_tail("sync", ch - 1)
            for el in _gate_elems("sync", ch):
                nc.sync.dma_start(el, tail)
            for dst, src in x_subs("sync", ch):
                nc.sync.dma_start(dst, src)

        def gpsimd_x(ch):
            tail = x_part_tail("gpsimd", ch - 1)
            for el in _gate_elems("gpsimd", ch):
                nc.gpsimd.tensor_copy(el, tail)
            for dst, src in x_subs("gpsimd", ch):
                nc.gpsimd.dma_start(dst, src)

        nc.scalar.dma_start(wqk, wqkd)
        nc.sync.dma_start(wv, wvd)
        nc.gpsimd.dma_start(mask, maskd)
        for dst, src in x_subs("scalar", 0):
            nc.scalar.dma_start(dst, src)
        for dst, src in x_subs("sync", 0):
            nc.sync.dma_start(dst, src)
        for dst, src in x_subs("gpsimd", 0):
            nc.gpsimd.dma_start(dst, src)

        qkT = persist.tile([P, T], BF, tag="qkT")   # qT rows 0:64, kT 64:128
        klo = persist.tile([H, T], BF, tag="klo")   # kT re-based at partition 0
        vaug = persist.tile([P, NT, VW], BF, tag="vaug")  # [v | 1] per s-tile
        ones = nc.const_aps.scalar_like(1.0, vaug)
        nc.vector.tensor_copy(vaug[:, :, H : H + 1], ones.broadcast_to((P, NT, 1)))

        # x chains: all sync/gpsimd gates pass while those engines are
        # otherwise idle; scalar's chunk 1 gate passes before the first exp.
        # scalar's chunk 2/3 dmas are emitted later, keyed into the exp
        # stream.  sync's epilogue-out dmas are pinned behind its x chain.
        scalar_x(1)
        for ch in (1, 2, 3):
            sync_x(ch)
            gpsimd_x(ch)

        def proj_qk(ch):
            chs = slice(ch * CH, (ch + 1) * CH)
            order = _c_order(ch)
            qk_ps = proj_ps.tile([P, CH], F32, tag="ps")
            for n, c in enumerate(order):
                nc.tensor.matmul(
                    qk_ps, wqk[:, c, :], xT[:, c, chs],
                    start=(n == 0), stop=(n == NC - 1),
                )
            nc.vector.tensor_copy(qkT[:, chs], qk_ps)

        def k_rebase(ch, half=None):
            # klo[:, cols] = id64.T @ qkT[64:128, cols] via PE (512 cycles),
            # then DVE cast from psum.  No partition constraint is violated:
            # lhsT and rhs both start at partition 64, out at partition 0.
            lo = ch * CH if half in (None, 0) else ch * CH + CH // 2
            w = CH if half is None else CH // 2
            k_ps = proj_ps.tile([H, w], F32, tag="ps", name=f"k_ps{ch}_{half}")
            nc.tensor.matmul(k_ps, id64, qkT[64:P, lo : lo + w], start=True, stop=True)
            nc.vector.tensor_copy(klo[:, lo : lo + w], k_ps)

        def proj_v(ch):
            v_ps = proj_ps.tile([P, TPC, H], F32, tag="ps")
            for j in range(TPC):
                s = TPC * ch + j
                for c in range(NC):
                    nc.tensor.matmul(
                        v_ps[:, j, :],
                        xT[:, c, s * P : (s + 1) * P],
                        wv[:, c, :],
                        start=(c == 0),
                        stop=(c == NC - 1),
                    )
            nc.vector.tensor_copy(vaug[:, TPC * ch : TPC * ch + TPC, 0:H], v_ps)

        def emit_scores(ch, i, defer_muls=False):
            """Score matmuls + merged exp for s-tile pair (2i, 2i+1) of chunk ch."""
            base = ch * CH
            wei = wei_ps.tile([P, 2, CH], F32, tag="w")
            cols = []
            for u in range(2):
                s = 2 * i + u
                diag = s >= TPC * ch
                col0 = (s - TPC * ch) * P if diag else 0
                cols.append(col0)
                nc.tensor.matmul(
                    wei[:, u, col0:],
                    klo[:, s * P : (s + 1) * P],
                    qkT[0:H, base + col0 : base + CH],
                    start=True,
                    stop=True,
                )
            cmin = min(cols)
            pT = pt_p.tile([P, 2, CH], BF)
            # one ACT instruction covers both s-tiles; cols [cmin:col0) of a
            # diagonal tile hold exp(stale psum) — finite and never read.
            nc.scalar.activation(
                pT[:, :, cmin:], wei[:, :, cmin:], Exp, scale=float(H) ** -0.5
            )

            def muls():
                for u in range(2):
                    s = 2 * i + u
                    if s >= TPC * ch:
                        c0 = cols[u]
                        nc.vector.tensor_mul(
                            pT[:, u, c0 : c0 + P], pT[:, u, c0 : c0 + P], tri
                        )

            if not defer_muls:
                muls()
                muls = None
            return [(2 * i, cols[0], pT, 0), (2 * i + 1, cols[1], pT, 1)], muls

        def emit_pv(ch, o_ps, s, col0, pT, u):
            # start=True clears has_written for the WHOLE psum bank, so only
            # the first matmul of the chunk may set it; later slices' first
            # writes land on cleared bits and overwrite, then accumulate.
            for j in range(col0 // P, TPC):
                tj = TPC * ch + j
                nc.tensor.matmul(
                    o_ps[:, j, :],
                    pT[:, u, j * P : (j + 1) * P],
                    vaug[:, s, 0 : H + 1],
                    start=(s == 0 and j == 0),
                    stop=(s == tj),
                    skip_group_check=True,
                )

        def epilogue(ch, o_ps, jlo=0, jhi=TPC, engines=None):
            n = jhi - jlo
            rc = rc_p.tile([P, n, 1], F32)
            nc.vector.reciprocal(rc, o_ps[:, jlo:jhi, H : H + 1])
            ost = ost_p.tile([P, n, H], F32)
            nc.vector.tensor_mul(
                ost, o_ps[:, jlo:jhi, 0:H], rc.broadcast_to((P, n, H))
            )
            t0 = ch * CH + jlo * P
            if engines is None:
                nc.sync.dma_start(
                    outd[t0 : t0 + n * P, :].rearrange("(n p) h -> p n h", p=P), ost
                )
            else:
                # split across queues for the latency-critical tail
                for jj, eng in zip(range(n), engines):
                    tj = t0 + jj * P
                    eng.dma_start(
                        outd[tj : tj + P, :].rearrange("(n p) h -> p n h", p=P),
                        ost[:, jj : jj + 1, :],
                    )

        # Emission: scores/exp stream ahead, PV of chunk ch interleaves with
        # proj of chunk ch+1 so neither PE nor ACT starves.
        o_ps = {}
        pend = []

        def drain(n):
            while len(pend) > n:
                ch_, s_, c0_, pT_, u_ = pend.pop(0)
                emit_pv(ch_, o_ps[ch_], s_, c0_, pT_, u_)
                if ch_ == NCH - 1 and s_ == TPC * ch_ + TPC - 2:
                    # second-to-last s-tile: t-tiles 0..1 of the last chunk
                    # are complete
                    epilogue(ch_, o_ps[ch_], 0, 2)
                elif s_ == TPC * ch_ + TPC - 1:
                    if ch_ == NCH - 1:
                        epilogue(
                            ch_, o_ps.pop(ch_), 2, TPC,
                            engines=[nc.gpsimd, nc.scalar],
                        )
                    else:
                        epilogue(ch_, o_ps.pop(ch_))

        proj_qk(0)
        o_ps[0] = o_ps_p.tile([P, TPC, H + 1], F32, tag="o", name="o_ps0")
        k_rebase(0, 0)
        # first score pair issues between the two k-rebase halves; its
        # mask-muls (which wait on the first exp) are emitted after the
        # second rebase cast so they don't block it in vector program order.
        p00, muls00 = emit_scores(0, 0, defer_muls=True)
        k_rebase(0, 1)
        p01, _ = emit_scores(0, 1)
        # chunk 2's scalar-queue dma, keyed on pair (0,1)'s pT so the
        # scheduler can't hoist its gate ahead of the first exps
        scalar_x(2, key=p01[0][2][0:1, 0, p01[0][1] : p01[0][1] + 1])
        muls00()
        pend.extend((0, *e) for e in p00)
        pend.extend((0, *e) for e in p01)
        proj_qk(1)
        k_rebase(1)
        # pin sync's epilogue-out dmas behind its whole x chain
        nc.sync.dma_start(gate_scr[0:1, 4:5], x_part_tail("sync", 3))
        for ch in range(1, NCH):
            o_ps[ch] = o_ps_p.tile([P, TPC, H + 1], F32, tag="o", name=f"o_ps{ch}")
            npairs = (TPC * ch + TPC) // 2
            for i in range(npairs):
                e, _ = emit_scores(ch, i)
                pend.extend((ch, *x) for x in e)
                if ch == 1 and i == 0:
                    proj_v(0)
                    scalar_x(3, key=e[0][2][0:1, 0, e[0][1] : e[0][1] + 1])
                if i == 2 * ch and ch + 1 < NCH:
                    proj_qk(ch + 1)
                    k_rebase(ch + 1)
                drain(3)
                if i == 2:
                    proj_v(ch)
        drain(0)

    nc.compile()
    return nc


_NC = None


def kernel(x, Wq, Wk, Wv, **run_kwargs):
    global _NC
    if _NC is None:
        _NC = build_kernel()
    x = np.asarray(x, dtype=np.float32)
    wqk = np.concatenate(
        [np.asarray(Wq, np.float32), np.asarray(Wk, np.float32)], axis=1
    ).astype(BF_NP)
    wqk_t = np.ascontiguousarray(wqk.reshape(NC, P, P).transpose(1, 0, 2))
    wv_t = np.ascontiguousarray(
        np.asarray(Wv, np.float32).astype(BF_NP).reshape(NC, P, H).transpose(1, 0, 2)
    )
    mask_np = np.zeros((P, P + H), dtype=BF_NP)
    mask_np[:, 0:P] = np.triu(np.ones((P, P), dtype=BF_NP))
    mask_np[64:P, P : P + H] = np.eye(H, dtype=BF_NP)
    in_maps = []
    for b in range(B):
        xT = x[b].T.astype(BF_NP)  # [C, T]
        # chunk-major tiled layout: [NCH, P, NC, CH], 8 KiB contiguous lines
        xT_t = np.ascontiguousarray(
            xT.reshape(NC, P, NCH, CH).transpose(2, 1, 0, 3)
        )
        in_maps.append({"xT": xT_t, "Wqk": wqk_t, "Wv": wv_t, "mask": mask_np})
    res = run_bass_kernel_spmd(_NC, in_maps, core_ids=list(range(B)), **run_kwargs)
    out = np.stack([res.results[b]["out"] for b in range(B)])
    if run_kwargs:
        kernel.last_result = res
    return out


if __name__ == "__main__":
    rng = np.random.default_rng(0)
    ins = {
        "x": rng.standard_normal((B, T, C), dtype=np.float32),
        "Wq": rng.standard_normal((C, H), dtype=np.float32) / np.sqrt(C),
        "Wk": rng.standard_normal((C, H), dtype=np.float32) / np.sqrt(C),
        "Wv": rng.standard_normal((C, H), dtype=np.float32) / np.sqrt(C),
    }
    out = kernel(**ins)
    print("out", out.shape, out.dtype)


# revision 18
# speedup vs baseline: 1.2968x; 1.2968x over previous
"""Single-head causal attention on 8 NeuronCores (batch-parallel), bf16.

x [8, 2048, 1024], Wq/Wk/Wv [1024, 64] -> out [8, 2048, 64].
One batch element per core. The host pre-transposes x to x.T (chunk-major
layout) and casts everything to bf16 (zero-flop marshalling), so the
device does no transposes at all:

  qkT[:,t]   = [Wq|Wk].T @ xT[:,t]      (qT rows 0:64, kT rows 64:128)
  v[t,:]     = xT[:,t-tile].T @ Wv      (natural [t,h] layout, PE direct)
  weiT[s,t]  = k[s]. q[t]              (lhsT = kT tile, rhs = qT cols)
  pT         = exp(weiT / sqrt(H))      (ACT, f32 psum -> bf16 sbuf,
                                         two s-tiles per instruction)
  out[t,h]   = sum_s pT[s,t] vaug[s,h]  (natural PV; ones column gives
                                         softmax denominators)
  out[t,h]  /= out[t,64]               (DVE reciprocal + scalar mul)

Causality via tile skipping, column-restricted diagonal score matmuls,
and one [128,128] triangular bf16 mask on diagonal blocks.  x.T chunks
are DMA'd through three initiating engines (sync/scalar/gpsimd) so the
first chunk lands early; emission interleaves proj(ch+1) with chunk ch's
PV stream so the scalar engine's exp pipeline never starves.
"""

from contextlib import ExitStack

import ml_dtypes
import numpy as np

import concourse.bass as bass
import concourse.mybir as mybir
import concourse.tile as tile
from concourse import bacc
from concourse.bass_utils import run_bass_kernel_spmd
from concourse.masks import make_upper_triangular

B, T, C, H = 8, 2048, 1024, 64
P = 128                      # partition tile
NT = T // P                  # 16 row tiles
NC = C // P                  # 8 contraction tiles
CH = 512                     # t-chunk width (psum bank)
NCH = T // CH                # 4 chunks
TPC = CH // P                # 4 t-tiles per chunk
VW = 66                      # vaug row stride: [v(64) | 1 | pad]

BF = mybir.dt.bfloat16
F32 = mybir.dt.float32
BF_NP = ml_dtypes.bfloat16

Exp = mybir.ActivationFunctionType.Exp


def build_kernel():
    nc = bacc.Bacc(
        "TRN2",
        target_bir_lowering=False,
        debug=False,
        enable_asserts=False,
        num_devices=B,
    )
    xTd = nc.dram_tensor("xT", [NCH, P, NC, CH], BF, kind="ExternalInput").ap()
    wqkd = nc.dram_tensor("Wqk", [P, NC, P], BF, kind="ExternalInput").ap()
    wvd = nc.dram_tensor("Wv", [P, NC, H], BF, kind="ExternalInput").ap()
    outd = nc.dram_tensor("out", [T, H], F32, kind="ExternalOutput").ap()

    with tile.TileContext(nc) as tc, ExitStack() as ctx:
        const = ctx.enter_context(tc.tile_pool(name="const", bufs=1))
        persist = ctx.enter_context(tc.tile_pool(name="persist", bufs=1))
        pt_p = ctx.enter_context(tc.tile_pool(name="pt", bufs=20))
        ost_p = ctx.enter_context(tc.tile_pool(name="ost", bufs=4))
        rc_p = ctx.enter_context(tc.tile_pool(name="rc", bufs=4))
        proj_ps = ctx.enter_context(tc.tile_pool(name="projps", bufs=2, space="PSUM"))
        wei_ps = ctx.enter_context(tc.tile_pool(name="weips", bufs=2, space="PSUM"))
        o_ps_p = ctx.enter_context(tc.tile_pool(name="ops", bufs=2, space="PSUM"))

        # x.T: chunks split across scalar+gpsimd DMA queues (sync reserved for
        # small latency-critical transfers: wqk, kTlo, out).  The DMA engines
        # round-robin across ALL enqueued transfers, so chunk ch+1 is only
        # enqueued once chunk ch has landed (1-element dummy reads gate the
        # sequencers); otherwise chunk 0's tail packets finish last and the
        # whole pipeline start slips by ~7us.
        # PE p-state warmup: matmuls on a zeroed scratch tile (no DMA
        # dependency) keep the tensor engine running during the x DMA wait so
        # the clock has ramped to 2.4 GHz before the first real projection.
        garbage = const.tile([P, CH], BF, tag="garbage")
        nc.vector.memset(garbage, 0.0)
        # 28 warmup matmuls keep the PE busy until chunk 0's completion
        # semaphore (~16us): any PE idle gap >2.4us here triggers a downclock
        # to half speed for >=3.4us, which would drag the first projections
        # AND the scalar engine's exp throughput (the DVFS couples engines).
        warm_ps = proj_ps.tile([P, CH], F32, tag="ps")
        for _ in range(28):
            nc.tensor.matmul(warm_ps, garbage[:, 0:P], garbage, start=True, stop=True)

        # x load plan (best measured configuration): chunk 0 split across all
        # three DMA-initiating engines, chunk 1 enqueued on sync at t0 (round-
        # robin with chunk 0 costs ~1us but lands chunk 1 early), chunks 2/3
        # split scalar+gpsimd with staggered issue - chunk 2 gated on chunk 0,
        # chunk 3's scalar side effectively on chunk 2's completion and its
        # gpsimd side on chunk 1.  The scalar-hosted gates delay the first exp
        # ~2us, but the early transfer issue keeps every later chunk ahead of
        # the exp stream, which measures faster than any non-blocking chain.
        xT = persist.tile([P, NC, T], BF, tag="xT")  # x.T: [c, t]
        wqk = const.tile([P, NC, P], BF, tag="wqk")
        nc.sync.dma_start(wqk, wqkd)
        wv = const.tile([P, NC, H], BF, tag="wv")
        nc.scalar.dma_start(wv, wvd)
        nc.scalar.dma_start(xT[:, 0:3, 0:CH], xTd[0][:, 0:3, :])
        nc.gpsimd.dma_start(xT[:, 3:6, 0:CH], xTd[0][:, 3:6, :])
        nc.sync.dma_start(xT[:, 6:8, 0:CH], xTd[0][:, 6:8, :])
        nc.sync.dma_start(xT[:, :, CH : 2 * CH], xTd[1])
        for ch in (2, 3):
            c0 = ch * CH
            g0 = c0 - 2 * CH
            nc.scalar.activation(
                xT[0:1, 0, c0 : c0 + 1],
                xT[0:1, 0, g0 + CH - 1 : g0 + CH],
                mybir.ActivationFunctionType.Copy,
            )
            nc.scalar.dma_start(xT[:, 0:4, c0 : c0 + CH], xTd[ch][:, 0:4, :])
            nc.gpsimd.tensor_copy(
                xT[0:1, 4, c0 : c0 + 1],
                xT[0:1, 5 if ch == 2 else 7, g0 + CH - 1 : g0 + CH],
            )
            nc.gpsimd.dma_start(xT[:, 4:8, c0 : c0 + CH], xTd[ch][:, 4:8, :])

        # causal mask: gpsimd writes f32; DVE copy converts to bf16
        scr_t = const.tile([P, P], F32, tag="scr_t")
        make_upper_triangular(nc, scr_t, val=1.0, diag=True)
        tri = const.tile([P, P], BF, tag="tri")  # tri[p,j]=1 iff j>=p
        nc.vector.tensor_copy(tri, scr_t)

        qkT = persist.tile([P, T], BF, tag="qkT")    # qT rows 0:64, kT 64:128
        kTlo = persist.tile([H, T], BF, tag="kTlo")  # kT re-based at partition 0
        vaug = persist.tile([P, NT, VW], BF, tag="vaug")  # [v | 1] per s-tile
        ones = nc.const_aps.scalar_like(1.0, vaug)
        nc.vector.tensor_copy(vaug[:, :, H : H + 1], ones.broadcast_to((P, NT, 1)))

        def proj_qk(ch):
            chs = slice(ch * CH, (ch + 1) * CH)
            qk_ps = proj_ps.tile([P, CH], F32, tag="ps")
            for c in range(NC):
                nc.tensor.matmul(
                    qk_ps, wqk[:, c, :], xT[:, c, chs], start=(c == 0), stop=(c == NC - 1)
                )
            nc.vector.tensor_copy(qkT[0:P if ch else H, chs], qk_ps[0:P if ch else H, :])
            if ch == 0:
                # chunk 0's attention starts right after this projection; an
                # extra PE k-projection into partitions 0:64 avoids waiting on
                # the SBUF->SBUF rebasing DMA's ~2.5us latency.
                k0_ps = proj_ps.tile([H, CH], F32, tag="ps", name="k0_ps")
                for c in range(NC):
                    nc.tensor.matmul(
                        k0_ps, wqk[:, c, H:P], xT[:, c, chs],
                        start=(c == 0), stop=(c == NC - 1),
                    )
                nc.vector.tensor_copy(kTlo[:, chs], k0_ps)
            else:
                nc.sync.dma_start(kTlo[:, chs], qkT[H:P, chs])

        def proj_v(ch):
            v_ps = proj_ps.tile([P, TPC, H], F32, tag="ps")
            for j in range(TPC):
                s = TPC * ch + j
                for c in range(NC):
                    nc.tensor.matmul(
                        v_ps[:, j, :],
                        xT[:, c, s * P : (s + 1) * P],
                        wv[:, c, :],
                        start=(c == 0),
                        stop=(c == NC - 1),
                    )
            nc.vector.tensor_copy(vaug[:, TPC * ch : TPC * ch + TPC, 0:H], v_ps)

        def emit_scores(ch, i):
            """Score matmuls + merged exp for s-tile pair (2i, 2i+1) of chunk ch."""
            base = ch * CH
            wei = wei_ps.tile([P, 2, CH], F32, tag="w")
            cols = []
            for u in range(2):
                s = 2 * i + u
                diag = s >= TPC * ch
                col0 = (s - TPC * ch) * P if diag else 0
                cols.append(col0)
                nc.tensor.matmul(
                    wei[:, u, col0:],
                    kTlo[:, s * P : (s + 1) * P],
                    qkT[0:H, base + col0 : base + CH],
                    start=True,
                    stop=True,
                )
            cmin = min(cols)
            pT = pt_p.tile([P, 2, CH], BF)
            # one ACT instruction covers both s-tiles; cols [cmin:col0) of a
            # diagonal tile hold exp(stale psum) — finite and never read.
            nc.scalar.activation(
                pT[:, :, cmin:], wei[:, :, cmin:], Exp, scale=float(H) ** -0.5
            )
            for u in range(2):
                s = 2 * i + u
                if s >= TPC * ch:
                    c0 = cols[u]
                    nc.vector.tensor_mul(
                        pT[:, u, c0 : c0 + P], pT[:, u, c0 : c0 + P], tri
                    )
            return [(2 * i, cols[0], pT, 0), (2 * i + 1, cols[1], pT, 1)]

        def emit_pv(ch, o_ps, s, col0, pT, u):
            # start=True clears has_written for the WHOLE psum bank, so only
            # the first matmul of the chunk may set it; later slices' first
            # writes land on cleared bits and overwrite, then accumulate.
            for j in range(col0 // P, TPC):
                tj = TPC * ch + j
                nc.tensor.matmul(
                    o_ps[:, j, :],
                    pT[:, u, j * P : (j + 1) * P],
                    vaug[:, s, 0 : H + 1],
                    start=(s == 0 and j == 0),
                    stop=(s == tj),
                    skip_group_check=True,
                )


        def epilogue(ch, o_ps, jlo=0, jhi=TPC, engines=None):
            n = jhi - jlo
            rc = rc_p.tile([P, n, 1], F32)
            nc.vector.reciprocal(rc, o_ps[:, jlo:jhi, H : H + 1])
            ost = ost_p.tile([P, n, H], F32)
            nc.vector.tensor_mul(
                ost, o_ps[:, jlo:jhi, 0:H], rc.broadcast_to((P, n, H))
            )
            t0 = ch * CH + jlo * P
            if engines is None:
                nc.gpsimd.dma_start(
                    outd[t0 : t0 + n * P, :].rearrange("(n p) h -> p n h", p=P), ost
                )
            else:
                # split across queues for the latency-critical tail
                for jj, eng in zip(range(n), engines):
                    tj = t0 + jj * P
                    eng.dma_start(
                        outd[tj : tj + P, :].rearrange("(n p) h -> p n h", p=P),
                        ost[:, jj : jj + 1, :],
                    )

        # Emission: scores/exp stream ahead, PV of chunk ch interleaves with
        # proj of chunk ch+1 so neither PE nor ACT starves.
        o_ps = {}
        pend = []

        def drain(n):
            while len(pend) > n:
                ch_, s_, c0_, pT_, u_ = pend.pop(0)
                emit_pv(ch_, o_ps[ch_], s_, c0_, pT_, u_)
                if ch_ == NCH - 1 and s_ == TPC * ch_ + TPC - 2:
                    # second-to-last s-tile: t-tiles 0..1 of the last chunk
                    # are complete; drain their output early
                    epilogue(ch_, o_ps[ch_], 0, 2)
                elif s_ == TPC * ch_ + TPC - 1:
                    if ch_ == NCH - 1:
                        epilogue(
                            ch_, o_ps.pop(ch_), 2, TPC,
                            engines=[nc.sync, nc.scalar],
                        )
                    else:
                        epilogue(ch_, o_ps.pop(ch_))

        proj_qk(0)
        proj_qk(1)
        for ch in range(NCH):
            o_ps[ch] = o_ps_p.tile([P, TPC, H + 1], F32, tag="o", name=f"o_ps{ch}")
            npairs = (TPC * ch + TPC) // 2
            for i in range(npairs):
                for e in emit_scores(ch, i):
                    pend.append((ch, *e))
                if i == 0:
                    proj_v(ch)
                if ch >= 1 and i == 2 * ch and ch + 1 < NCH:
                    proj_qk(ch + 1)
                drain(3)
        drain(0)

    nc.compile()
    return nc


_NC = None


def kernel(x, Wq, Wk, Wv, **run_kwargs):
    global _NC
    if _NC is None:
        _NC = build_kernel()
    x = np.asarray(x, dtype=np.float32)
    wqk = np.concatenate(
        [np.asarray(Wq, np.float32), np.asarray(Wk, np.float32)], axis=1
    ).astype(BF_NP)
    wqk_t = np.ascontiguousarray(wqk.reshape(NC, P, P).transpose(1, 0, 2))
    wv_t = np.ascontiguousarray(
        np.asarray(Wv, np.float32).astype(BF_NP).reshape(NC, P, H).transpose(1, 0, 2)
    )
    in_maps = []
    for b in range(B):
        xT = x[b].T.astype(BF_NP)  # [C, T]
        # chunk-major tiled layout: [NCH, P, NC, CH], 8 KiB contiguous lines
        xT_t = np.ascontiguousarray(
            xT.reshape(NC, P, NCH, CH).transpose(2, 1, 0, 3)
        )
        in_maps.append({"xT": xT_t, "Wqk": wqk_t, "Wv": wv_t})
    res = run_bass_kernel_spmd(_NC, in_maps, core_ids=list(range(B)), **run_kwargs)
    out = np.stack([res.results[b]["out"] for b in range(B)])
    if run_kwargs:
        kernel.last_result = res
    return out


if __name__ == "__main__":
    rng = np.random.default_rng(0)
    ins = {
        "x": rng.standard_normal((B, T, C), dtype=np.float32),
        "Wq": rng.standard_normal((C, H), dtype=np.float32) / np.sqrt(C),
        "Wk": rng.standard_normal((C, H), dtype=np.float32) / np.sqrt(C),
        "Wv": rng.standard_normal((C, H), dtype=np.float32) / np.sqrt(C),
    }
    out = kernel(**ins)
    print("out", out.shape, out.dtype)

